# revision 26
# baseline (speedup 1.0000x reference)
"""7x7 'same' 2D convolution over [128, 512, 512] f32, data-parallel on 8 NeuronCores.

Banded-Toeplitz formulation on the TensorEngine with 64x64 array
packing: the PE array runs as 4 independent 64x64 tiles
(tile_position=(64r, 64g)), each computing a 58-row output block
    out[i0+m, j] = sum_v sum_{r'} T_v[r', m] * xpad[i0+r', j+v]
with T_v[r', m] = w[r'-m, v] (band, r'<64, m<58). The 7 column taps (v)
accumulate into PSUM; tile (s, r2, g2) covers out rows
232s + 116g2 + 58r2 + [0,58) and writes PSUM bank (s, r2), partitions
[64g2, 64g2+58). One 4-tile tap sweep streams in one N=512 matmul
time; 8 tiles cover rows 0..463 of an image. Rows 464..511 of four
consecutive images batch into one 4-tile "tail" group.

Inputs are cast to fp16 and pre-staged host-side into the SBUF slab
layout (partition 64r+p, slab q, col) = padded row 116q + 58r + p
(slab 4 = tail rows, duplicated on both strips), so each image loads
with a single contiguous 663KB DMA. Accumulation is fp32; outputs are
stored as raw bf16 PSUM-bank dumps and un-permuted on the host.
PSUM evacuation alternates VectorE / ScalarE; stores alternate the
scalar HWDGE ring and the gpsimd SWDGE ring; loads own the sync ring.
"""

import numpy as np

B, H, W = 128, 512, 512
KS = 7
PAD = (KS - 1) // 2          # 3
HP = H + 2 * PAD             # 518
N_CORES = 8
PER_CORE = B // N_CORES      # 16
TS = 58                      # output rows per 64x64 tile (64 - 6)
NS = 5                       # slabs per image (4 group-1 + 1 tail)
TAILM = H - 8 * TS           # 48 tail output rows per image
TAILK = TAILM + KS - 1       # 54


def _build_program():
    import concourse.bass as bass
    import concourse.tile as tile
    from concourse import bacc, mybir

    f16 = mybir.dt.float16
    bf16 = mybir.dt.bfloat16
    f32 = mybir.dt.float32

    nc = bacc.Bacc("TRN2", target_bir_lowering=False, debug=False,
                   num_devices=N_CORES)
    x_ext = nc.declare_dram_parameter("x", [PER_CORE, 128, NS * HP], f16,
                                      isOutput=False)
    t_ext = nc.declare_dram_parameter("toep", [128, KS * TS], f16,
                                      isOutput=False)
    # og[img, s, r] = dump of PSUM bank (s, r):
    #   row 64g+p  ->  out row 232s + 116g + 58r + p   (valid p < 58)
    og_ext = nc.declare_dram_parameter("og", [PER_CORE, 2, 2, 128, W],
                                       bf16, isOutput=True)
    # ot[tg, r] = tail bank dump: row 64g+p -> img 4tg + 2g + r,
    #   out row 464 + p  (valid p < 48)
    ot_ext = nc.declare_dram_parameter("ot", [PER_CORE // 4, 2, 128, W],
                                       bf16, isOutput=True)

    with tile.TileContext(nc) as tc:
        with (
            tc.tile_pool(name="toep", bufs=1) as toep_pool,
            tc.tile_pool(name="xin", bufs=6) as x_pool,
            tc.tile_pool(name="psum", bufs=8, space="PSUM") as psum_pool,
            tc.tile_pool(name="outs", bufs=8) as out_pool,
        ):
            toep_sb = toep_pool.tile([128, KS * TS], f16)
            nc.gpsimd.dma_start(out=toep_sb[:], in_=t_ext[:])

            def evac(ps, dst, idx):
                o_sb = out_pool.tile([128, W], bf16, name="o", tag="osb")
                if idx % 2 == 0:
                    nc.vector.tensor_copy(o_sb[:], ps[:])
                else:
                    nc.scalar.copy(o_sb[:], ps[:])
                ring = nc.scalar if idx % 4 < 2 else nc.gpsimd
                ring.dma_start(out=dst, in_=o_sb[:])

            stages = {}
            for img in range(PER_CORE):
                # (partition 64r+p, slab q) = padded row 116q + 58r + p
                stage = x_pool.tile([128, NS * HP], f16, name="stage",
                                    tag="stage")
                # slabs 0-1 arrive first (all the s=0 group needs)
                nc.sync.dma_start(out=stage[:, :2 * HP],
                                  in_=x_ext[img, :, :2 * HP])
                nc.sync.dma_start(out=stage[:, 2 * HP:],
                                  in_=x_ext[img, :, 2 * HP:])
                stages[img] = stage

                # s-groups sequential: only 2 PSUM banks live per group,
                # so allocation never stalls on evacuation of 4 banks.
                for s in range(2):
                    ps = [psum_pool.tile([128, W], f32, name=f"ps{r}",
                                         tag="acc") for r in range(2)]
                    for v in range(KS):
                        for g in range(2):
                            for r in range(2):
                                q = 2 * s + g
                                nc.tensor.matmul(
                                    ps[r][64 * g:64 * g + TS, :],
                                    toep_sb[64 * r:64 * r + 64,
                                            TS * v:TS * (v + 1)],
                                    stage[64 * r:64 * r + 64,
                                          q * HP + v:q * HP + v + W],
                                    start=(v == 0),
                                    stop=(v == KS - 1),
                                    tile_position=(64 * r, 64 * g),
                                )
                    for r in range(2):
                        evac(ps[r], og_ext[img, s, r], img * 4 + 2 * s + r)

                if img % 4 == 3:
                    tg = img // 4
                    pst = [psum_pool.tile([128, W], f32, name=f"pt{r}",
                                          tag="acc") for r in range(2)]
                    for v in range(KS):
                        for j in range(4):
                            r, g = j % 2, j // 2
                            nc.tensor.matmul(
                                pst[r][64 * g:64 * g + TAILM, :],
                                toep_sb[64 * r:64 * r + TAILK,
                                        TS * v:TS * v + TAILM],
                                stages[4 * tg + j][64 * r:64 * r + TAILK,
                                                   4 * HP + v:4 * HP + v + W],
                                start=(v == 0),
                                stop=(v == KS - 1),
                                tile_position=(64 * r, 64 * g),
                            )
                    for r in range(2):
                        evac(pst[r], ot_ext[tg, r], img * 4 + r)
                    stages = {}
    nc.finalize()
    return nc


def _host_prep(x, w):
    x = np.asarray(x, dtype=np.float32)
    w = np.asarray(w, dtype=np.float32)
    # padded images with extra zero rows (slab-4 strip-1 reads to 585)
    xpad = np.zeros((B, 586, HP), dtype=np.float16)
    xpad[:, PAD:PAD + H, PAD:PAD + W] = x
    # slab layout: (p, q) -> padded row 116q + 58*(p//64) + p%64;
    # slab 4 = tail rows 464+, duplicated on both 64-row strips
    p = np.arange(128)
    q = np.arange(NS)
    ridx = 116 * q[None, :] + 58 * (p[:, None] // 64) + (p[:, None] % 64)
    ridx[:, 4] = 464 + (p % 64)
    xslab = np.ascontiguousarray(
        xpad[:, ridx, :].reshape(B, 128, NS * HP))
    # Toeplitz band [64, 58] per tap, replicated on both partition strips
    toep = np.zeros((128, KS * TS), dtype=np.float16)
    w16 = w.astype(np.float16)
    idx = np.arange(TS)
    for st in range(2):
        for v in range(KS):
            for d in range(KS):
                toep[64 * st + idx + d, TS * v + idx] = w16[d, v]
    return xslab, toep


def _execute(x, w, **run_kwargs):
    from concourse.bass_utils import run_bass_kernel_spmd

    xslab, toep = _host_prep(x, w)
    nc = _build_program()
    in_maps = [
        {"x": xslab[c * PER_CORE:(c + 1) * PER_CORE], "toep": toep}
        for c in range(N_CORES)
    ]
    last_err = None
    for _attempt in range(3):
        try:
            res = run_bass_kernel_spmd(nc, in_maps,
                                       core_ids=list(range(N_CORES)),
                                       **run_kwargs)
            break
        except Exception as e:  # transient NRT execute flakes -> retry
            last_err = e
    else:
        raise last_err
    out = np.empty((B, H, W), dtype=np.float32)
    for c in range(N_CORES):
        sl = slice(c * PER_CORE, (c + 1) * PER_CORE)
        og = np.asarray(res.results[c]["og"], dtype=np.float32)
        ot = np.asarray(res.results[c]["ot"], dtype=np.float32)
        og6 = og.reshape(PER_CORE, 2, 2, 2, 64, W)[:, :, :, :, :TS, :]
        # [img, s, r, g, p, w] -> row = 232s + 116g + 58r + p
        out[sl, :8 * TS, :] = og6.transpose(0, 1, 3, 2, 4, 5).reshape(
            PER_CORE, 8 * TS, W)
        ot5 = ot.reshape(PER_CORE // 4, 2, 2, 64, W)[:, :, :, :TAILM, :]
        # [tg, r, g, p, w] -> img 4tg + 2g + r, row 464 + p
        out[sl, 8 * TS:, :] = ot5.transpose(0, 2, 1, 3, 4).reshape(
            PER_CORE, TAILM, W)
    return out, res


def kernel(x, w):
    out, _ = _execute(x, w)
    return out
